# revision 15
# baseline (speedup 1.0000x reference)
"""Trainium2 Bass kernel for AlignedTriLoss (global hard-mining triplet loss +
DMLI local-stripe shortest-path loss), SPMD over 8 NeuronCores.

Strategy (row-sharded mining, v4 — dma_gather-transpose DMLI path):
  * Host precomputes everything cheap on CPU: bf16 casts of gf/lf, squared
    norms (-0.5*sq as stacked bf16 hi+lo rows), one-hot label matrices
    (moving: [256,N] 1.0; stationary: [256,NA] -L/2), per-stripe local norms,
    and channel-major pre-transposed own-anchor local features.  No
    AllGather / barrier: each core is fully independent.
  * Each core owns N/8 anchor rows and computes, via a fused bf16 TensorE
    matmul chain with an augmented contraction dimension,
        P[i, j] = gf_i . gf_j  - 0.5*sq_j - 0.5*L*[t_i == t_j]
    so that  -2*P[i, j] + sq_i = d2[i, j] + L*eq[i, j].
    Row min/max of P (+ DVE max_index straight on PSUM) give the hardest
    positive / negative values *and* their column indices.  Columns are
    processed in quarters of 1024; within a quarter the 4 anchor tiles are
    split into two halves so the 8 PSUM banks double-buffer: the PE works on
    one half while the DVE drains the other.
  * DMLI: mined indices are rewrapped (int16, 16-partition layout) with one
    small DMA per anchor tile/side, then a single gpsimd dma_gather with
    transpose=True fetches the 128 hard rows of lf straight into
    channel-major [c, (stripe, chunk), anchor] layout (xbar transpose, no
    PE/DVE work).  Gram matmuls read that tile through strided moving APs
    (16-anchor groups), accumulate over the 2 channel chunks into PSUM, and
    the 8x8 block-diagonals are pulled out with one 4D-AP SBUF->SBUF DMA per
    tile/side.  Stripe norms of the mined rows come from a tiny indirect
    row-gather.  d2 assembly, sqrt+tanh and the 8x8 shortest-path DP run
    batched across all 8 (tile, side) lanes in single wide DVE/ACT ops.
Host side only shards/preprocesses inputs, concatenates per-core outputs and
takes means.
"""

import numpy as np
import ml_dtypes

import concourse.bass as bass
import concourse.bacc as bacc
import concourse.mybir as mybir
import concourse.tile as tile
from concourse.bass_utils import run_bass_kernel_spmd

F32 = mybir.dt.float32
U32 = mybir.dt.uint32
I16 = mybir.dt.int16
BF16 = mybir.dt.bfloat16
AF = mybir.ActivationFunctionType
ALU = mybir.AluOpType
AX = mybir.AxisListType

P = 128
MARGIN = 0.3
EPS = 1e-12
LBL = 16384.0   # label-match offset; > max d2 (~5k), exact in bf16
BIG = 1e30
BIGI = 65536.0


def _sub(base_ap, off, free_dims):
    """AP at base+off with explicit free dims (keeps base partition dim)."""
    return bass.AP(base_ap.tensor, base_ap.offset + off, [base_ap.ap[0]] + free_dims)


def _flat(base_ap, off, dims):
    """Fully flat AP (dims may cross partitions at the tile's row pitch)."""
    return bass.AP(base_ap.tensor, base_ap.offset + off, dims)


def build_program(N=4096, DG=2048, M=8, C=256, NC=8, NCLS=256):
    NA = N // NC            # anchors per core
    MT = NA // P            # anchor tiles per core
    KG = DG // P            # gf k-tiles
    OH = NCLS // P          # one-hot k-tiles
    CH = C // P             # 128-chunks per local stripe
    G16 = P // (2 * M)      # anchor groups per tile (8 groups of 16)
    NBW = 512               # mining psum tile width
    NQB = 2                 # psum tiles (column blocks) per anchor tile
    QW = NQB * NBW          # columns per quarter
    NQ = N // QW            # quarters
    NPART = NQ * NQB        # per-row partials
    MC = M * C
    NK = KG + OH + 1        # mining k rounds (gf, onehot, stacked sq hi/lo)
    SZ = (M + 1) * (M + 1)  # padded DP matrix size
    F = M * M
    GA = 2 * M              # anchors per gram group (16)
    NL = 2 * MT             # DMLI lanes: (mt, side)
    assert G16 * GA == P and C % P == 0 and MT % 2 == 0

    nc = bacc.Bacc("TRN2", target_bir_lowering=False, debug=False, num_devices=NC)

    gf_mov = nc.dram_tensor("gf_mov", [DG, N], BF16, kind="ExternalInput").ap()
    gf_st = nc.dram_tensor("gf_st", [DG, NA], BF16, kind="ExternalInput").ap()
    oh_mov_d = nc.dram_tensor("oh_mov", [NCLS, N], BF16, kind="ExternalInput").ap()
    oh_st_d = nc.dram_tensor("oh_st", [NCLS, NA], BF16, kind="ExternalInput").ap()
    sqhl_d = nc.dram_tensor("sqhl", [2, N], BF16, kind="ExternalInput").ap()
    sq_own_d = nc.dram_tensor("sq_own", [NA], F32, kind="ExternalInput").ap()
    lfb_d = nc.dram_tensor("lfb", [N, MC], BF16, kind="ExternalInput").ap()
    lftx_d = nc.dram_tensor("lftx", [P, MT * MC], BF16, kind="ExternalInput").ap()
    sql_d = nc.dram_tensor("sql", [N, M], F32, kind="ExternalInput").ap()
    sqxp_d = nc.dram_tensor("sqxp", [P, NL * F], F32, kind="ExternalInput").ap()
    out_d = nc.dram_tensor("out", [NA, 8], F32, kind="ExternalOutput").ap()

    with tile.TileContext(nc) as tc:
        cst = tc.alloc_tile_pool(name="cst", bufs=1)
        mine_p = tc.alloc_tile_pool(name="mine", bufs=1)

        # ---- stationary mining operands (lead the scalar DGE queue) ----
        st_p0 = tc.alloc_tile_pool(name="st", bufs=KG)
        st = []
        for k in range(KG):
            t = st_p0.tile([P, NA], BF16, tag="st", name=f"st{k}")
            nc.scalar.dma_start(t[:], gf_st[k * P:(k + 1) * P, :])
            st.append(t)
        oh_st = []
        for h in range(OH):
            t = cst.tile([P, NA], BF16, name=f"ohst{h}", tag=f"ohst{h}")
            nc.scalar.dma_start(t[:], oh_st_d[h * P:(h + 1) * P, :])
            oh_st.append(t)
        oh_mov = []
        for h in range(OH):
            t = cst.tile([P, N], BF16, name=f"ohm{h}", tag=f"ohm{h}")
            nc.gpsimd.dma_start(t[:], oh_mov_d[h * P:(h + 1) * P, :])
            oh_mov.append(t)
        sqh = cst.tile([2, N], BF16, name="sqh", tag="sqh")
        nc.gpsimd.dma_start(sqh[:], sqhl_d[:])
        ones2 = cst.tile([2, P], BF16)
        nc.gpsimd.memset(ones2[:], 1.0)
        sqo, txs = [], []
        for mt in range(MT):
            s = cst.tile([P, 1], F32, name=f"sqo{mt}", tag=f"sqo{mt}")
            nc.scalar.dma_start(s[:], sq_own_d[mt * P:(mt + 1) * P])
            sqo.append(s)
            t = cst.tile([P, MC], BF16, name=f"tx{mt}", tag=f"tx{mt}")
            nc.scalar.dma_start(t[:], lftx_d[:, mt * MC:(mt + 1) * MC])
            txs.append(t)
        sqxp = cst.tile([P, NL * F], F32, name="sqxp", tag="sqxp")
        nc.scalar.dma_start(sqxp[:], sqxp_d[:])

        sdr = tc.alloc_tile_pool(name="sdr", bufs=1, space="DRAM")
        idx_dr = [sdr.tile([P], I16, tag=f"idr{i}", name=f"idr{i}")
                  for i in range(NL)]

        pvq = [mine_p.tile([P, 2 * NPART], F32, tag=f"pvq{mt}", name=f"pvq{mt}")
               for mt in range(MT)]
        piq = [mine_p.tile([P, 2 * NPART], F32, tag=f"piq{mt}", name=f"piq{mt}")
               for mt in range(MT)]

        # ---- mining matmul ----
        with tc.tile_pool(name="minps", bufs=1, space="PSUM") as mmps, \
                tc.tile_pool(name="mov", bufs=KG + 2) as mov_p, \
                tc.tile_pool(name="mq", bufs=4) as mq_p:
            for q in range(NQ):
                qs = q * QW
                movf = []
                for k in range(KG):
                    t = mov_p.tile([P, QW], BF16, tag="movf", name=f"movf{k}")
                    nc.sync.dma_start(t[:], gf_mov[k * P:(k + 1) * P, qs:qs + QW])
                    movf.append(t)
                for half in range(2):
                    mts = (0, 1) if half == 0 else (2, 3)
                    pt = {}
                    for k in range(NK):
                        for mt in mts:
                            if k < KG:
                                lh = st[k][:, mt * P:(mt + 1) * P]
                                rhf = movf[k][:]
                            elif k < KG + OH:
                                lh = oh_st[k - KG][:, mt * P:(mt + 1) * P]
                                rhf = oh_mov[k - KG][:, qs:qs + QW]
                            else:
                                lh = ones2[:]
                                rhf = sqh[:, qs:qs + QW]
                            for nb in range(NQB):
                                if k == 0:
                                    pt[(mt, nb)] = mmps.tile(
                                        [P, NBW], F32, tag=f"mps{mt}_{nb}",
                                        name=f"mps{mt}_{nb}")
                                rh = rhf[:, nb * NBW:(nb + 1) * NBW]
                                nc.tensor.matmul(
                                    pt[(mt, nb)][:], lh, rh,
                                    start=(k == 0), stop=(k == NK - 1))
                    # drain this half's PSUM (overlaps other half's matmuls)
                    for mt in mts:
                        for nb in range(NQB):
                            ix = q * NQB + nb
                            ps = pt[(mt, nb)][:]
                            nc.vector.tensor_reduce(
                                pvq[mt][:, ix:ix + 1], ps, axis=AX.X, op=ALU.min)
                            nc.vector.tensor_reduce(
                                pvq[mt][:, NPART + ix:NPART + ix + 1], ps,
                                axis=AX.X, op=ALU.max)
                            i8a = mq_p.tile([P, 8], U32, tag="i8a", name=f"i8a{mt}")
                            i8b = mq_p.tile([P, 8], U32, tag="i8b", name=f"i8b{mt}")
                            nc.vector.max_index(
                                i8a[:], pvq[mt][:, ix:ix + 1].to_broadcast([P, 8]),
                                ps)
                            nc.vector.max_index(
                                i8b[:],
                                pvq[mt][:, NPART + ix:NPART + ix + 1]
                                .to_broadcast([P, 8]), ps)
                            off = float(qs + nb * NBW)
                            nc.gpsimd.tensor_copy(
                                piq[mt][:, ix:ix + 1], i8a[:, 0:1])
                            nc.gpsimd.tensor_copy(
                                piq[mt][:, NPART + ix:NPART + ix + 1], i8b[:, 0:1])
                            if off:
                                nc.gpsimd.tensor_scalar(
                                    piq[mt][:, ix:ix + 1], piq[mt][:, ix:ix + 1],
                                    off, None, op0=ALU.add)
                                nc.gpsimd.tensor_scalar(
                                    piq[mt][:, NPART + ix:NPART + ix + 1],
                                    piq[mt][:, NPART + ix:NPART + ix + 1],
                                    off, None, op0=ALU.add)

        # ---- combine partials: values, first-occurrence indices, loss ----
        out_tiles, apx_t, anx_t, idxw_t = [], [], [], []
        for mt in range(MT):
            ot = mine_p.tile([P, 8], F32, name=f"ot{mt}", tag=f"ot{mt}")
            out_tiles.append(ot)
            mn = mine_p.tile([P, 1], F32, tag=f"mn{mt}")
            mx = mine_p.tile([P, 1], F32, tag=f"mx{mt}")
            nc.vector.tensor_reduce(mn[:], pvq[mt][:, 0:NPART], axis=AX.X,
                                    op=ALU.min)
            nc.vector.tensor_reduce(mx[:], pvq[mt][:, NPART:2 * NPART],
                                    axis=AX.X, op=ALU.max)
            idxs, idxws = [], []
            for side, vref in ((0, mn), (1, mx)):
                msk = mine_p.tile([P, NPART], F32, tag=f"msk{mt}_{side}")
                nc.vector.tensor_tensor(
                    msk[:], pvq[mt][:, side * NPART:(side + 1) * NPART],
                    vref[:, 0:1].to_broadcast([P, NPART]), op=ALU.is_equal)
                cnd = mine_p.tile([P, NPART], F32, tag=f"cnd{mt}_{side}")
                nc.vector.tensor_scalar(
                    cnd[:], piq[mt][:, side * NPART:(side + 1) * NPART],
                    -BIGI, None, op0=ALU.add)
                nc.vector.tensor_tensor(cnd[:], cnd[:], msk[:], op=ALU.mult)
                nc.vector.tensor_scalar(cnd[:], cnd[:], BIGI, None, op0=ALU.add)
                sel = mine_p.tile([P, 1], F32, tag=f"sel{mt}_{side}")
                nc.vector.tensor_reduce(sel[:], cnd[:], axis=AX.X, op=ALU.min)
                iu = mine_p.tile([P, 1], U32, tag=f"iu{mt}_{side}")
                nc.vector.tensor_copy(iu[:], sel[:])
                nc.gpsimd.tensor_copy(ot[:, 4 + side:5 + side], sel[:])
                # int16 copy of the mined index, rewrapped via a DRAM bounce
                # to the 16-partition layout dma_gather wants (replicated to
                # partitions 16-31, the q7 tx core of swdge queue 0)
                iu16 = mine_p.tile([P, 1], I16, tag=f"i16_{mt}_{side}")
                nc.gpsimd.tensor_copy(iu16[:], sel[:])
                idr = idx_dr[mt * 2 + side]
                eng = nc.sync if side == 0 else nc.scalar
                eng.dma_start(idr[:], iu16[:])
                idxw = mine_p.tile([P, 8], I16, tag=f"ixw{mt}_{side}",
                                   name=f"ixw{mt}_{side}")
                nc.gpsimd.memset(idxw[:], 0.0)
                eng.dma_start(_flat(idxw[:], 0, [[8, 16], [1, 8]]),
                              _flat(idr[:], 0, [[1, 16], [16, 8]]))
                eng.dma_start(_flat(idxw[:], P, [[8, 16], [1, 8]]),
                              _flat(idxw[:], 0, [[8, 16], [1, 8]]))
                idxs.append(iu)
                idxws.append(idxw)
            apx_t.append(idxs[0])
            anx_t.append(idxs[1])
            idxw_t.append(idxws)
            # d2_ap = -2*mn - L + sq_i ; d2_an = -2*mx + sq_i
            t1 = mine_p.tile([P, 2], F32, tag=f"t1{mt}")
            nc.vector.tensor_scalar(t1[:, 0:1], mn[:], -2.0, -LBL,
                                    op0=ALU.mult, op1=ALU.add)
            nc.vector.tensor_scalar(t1[:, 1:2], mx[:], -2.0, None, op0=ALU.mult)
            d2 = mine_p.tile([P, 2], F32, tag=f"d2{mt}")
            nc.vector.tensor_tensor(
                d2[:], t1[:], sqo[mt][:, 0:1].to_broadcast([P, 2]), op=ALU.add)
            nc.vector.tensor_scalar(d2[:], d2[:], EPS, None, op0=ALU.max)
            nc.scalar.activation(ot[:, 2:4], d2[:], AF.Sqrt)
            gsub = mine_p.tile([P, 1], F32, tag=f"gs{mt}")
            nc.vector.tensor_tensor(gsub[:], ot[:, 2:3], ot[:, 3:4],
                                    op=ALU.subtract)
            nc.vector.tensor_scalar(ot[:, 0:1], gsub[:], MARGIN, 0.0,
                                    op0=ALU.add, op1=ALU.max)
            nc.gpsimd.tensor_copy(ot[:, 6:7], mn[:])
            nc.gpsimd.tensor_copy(ot[:, 7:8], mx[:])
        st_p0.release()

        # ---- DMLI local loss ----
        with tc.tile_pool(name="dml", bufs=1) as dml, \
                tc.tile_pool(name="gps", bufs=4, space="PSUM") as gps, \
                tc.tile_pool(name="gsb", bufs=4) as gsb_p, \
                tc.tile_pool(name="gdr", bufs=4, space="DRAM") as gdr, \
                tc.tile_pool(name="dp", bufs=1) as dpp:
            dt_all = dml.tile([P, NL * F], BF16, name="dtall", tag="dtall")
            sqys = {}
            for mt in range(MT):
                for side, iu in ((0, apx_t[mt]), (1, anx_t[mt])):
                    lane = mt * 2 + side
                    # stripe norms of the mined rows (tiny indirect gather)
                    sqy = dpp.tile([P, M], F32, tag=f"sqy{mt}{side}",
                                   name=f"sqy{mt}{side}")
                    nc.gpsimd.indirect_dma_start(
                        out=sqy[:], out_offset=None, in_=sql_d[:],
                        in_offset=bass.IndirectOffsetOnAxis(ap=iu[:, 0:1],
                                                            axis=0))
                    sqys[lane] = sqy
                    # gather+transpose the 128 hard lf rows: ty[c, (b,h), i]
                    ty = dml.tile([P, MC], BF16, tag=f"ty{lane}",
                                  name=f"ty{lane}")
                    nc.gpsimd.dma_gather(
                        out_ap=_sub(ty[:], 0, [[P, MC // P], [1, P]]),
                        in_ap=lfb_d[:],
                        idxs_ap=idxw_t[mt][side][:],
                        num_idxs=P,
                        num_idxs_reg=P,
                        elem_size=MC,
                        transpose=True,
                    )
                    # gram matmuls: 8 groups of 16 anchors, accumulate over
                    # the CH channel chunks; 4 groups share one PSUM bank
                    gpa = gps.tile([P, 4 * P], F32, tag="gramA",
                                   name=f"grA{lane}")
                    gpb = gps.tile([P, 4 * P], F32, tag="gramB",
                                   name=f"grB{lane}")
                    for g in range(G16):
                        gp = gpa if g < 4 else gpb
                        col = (g % 4) * P
                        for h in range(CH):
                            nc.tensor.matmul(
                                gp[:, col:col + P],
                                txs[mt][:, h * M * P + g * P:
                                         h * M * P + (g + 1) * P],
                                _sub(ty[:], h * P + g * GA,
                                     [[1, GA], [CH * P, M]]),
                                start=(h == 0), stop=(h == CH - 1))
                    gsb = gsb_p.tile([P, G16 * P], BF16, tag="gsb",
                                     name=f"gsb{lane}")
                    if side == 0:
                        nc.vector.tensor_copy(gsb[:, 0:4 * P], gpa[:])
                        nc.scalar.activation(gsb[:, 4 * P:8 * P], gpb[:],
                                             AF.Copy)
                    else:
                        nc.scalar.activation(gsb[:, 0:4 * P], gpa[:], AF.Copy)
                        nc.vector.tensor_copy(gsb[:, 4 * P:8 * P], gpb[:])
                    # bounce the gram tile to DRAM, then per-group 3D DMAs
                    # pull the 16 8x8 diag blocks into anchor-major dt:
                    # dt[p=(g,u), lane*F + a*M + b]
                    gd = gdr.tile([P * G16 * P], BF16, tag="gd",
                                  name=f"gd{lane}")
                    nc.sync.dma_start(gd[:], gsb[:])
                    for g in range(G16):
                        eng = (nc.sync, nc.scalar, nc.gpsimd)[(lane + g) % 3]
                        eng.dma_start(
                            _flat(dt_all[:], (g * GA) * (NL * F) + lane * F,
                                  [[NL * F, GA], [M, M], [1, M]]),
                            _flat(gd[:], g * P,
                                  [[M * G16 * P + M, GA],
                                   [G16 * P, M], [1, M]]))

            # ---- d2 assembly + sqrt/tanh + DP, batched over all 8 lanes ----
            u2 = dpp.tile([P, NL * F], F32, tag="u2", name="u2")
            nc.vector.tensor_scalar(u2[:], dt_all[:], -2.0, None, op0=ALU.mult)
            nc.vector.tensor_tensor(u2[:], u2[:], sqxp[:], op=ALU.add)
            for lane in range(NL):
                nc.vector.tensor_tensor(
                    u2[:, lane * F:(lane + 1) * F],
                    u2[:, lane * F:(lane + 1) * F],
                    _sub(sqys[lane][:], 0, [[0, M], [1, M]]), op=ALU.add)
            nc.vector.tensor_scalar(u2[:], u2[:], EPS, None, op0=ALU.max)
            nc.scalar.activation(u2[:], u2[:], AF.Sqrt)
            dist = dpp.tile([P, NL * SZ], F32, tag="dist", name="dist")
            nc.gpsimd.memset(dist[:], BIG)
            nc.gpsimd.memset(_sub(dist[:], 1, [[SZ, NL], [1, 1]]), 0.0)
            for lane in range(NL):
                nc.scalar.activation(
                    _sub(dist[:], lane * SZ + (M + 1) + 1,
                         [[M + 1, M], [1, M]]),
                    _sub(u2[:], lane * F, [[M, M], [1, M]]),
                    AF.Tanh, scale=0.5)
            tmp = dpp.tile([P, NL * M], F32, tag="dptmp")
            for k in range(2, 2 * M + 1):
                a_lo = max(1, k - M)
                n = min(M, k - 1) - a_lo + 1
                s1 = M * a_lo + k - (M + 1)
                s2 = M * a_lo + k - 1
                dst = M * a_lo + k
                nc.vector.tensor_tensor(
                    _sub(tmp[:], 0, [[M, NL], [1, n]]),
                    _sub(dist[:], s1, [[SZ, NL], [M, n]]),
                    _sub(dist[:], s2, [[SZ, NL], [M, n]]), op=ALU.min)
                nc.vector.tensor_tensor(
                    _sub(dist[:], dst, [[SZ, NL], [M, n]]),
                    _sub(tmp[:], 0, [[M, NL], [1, n]]),
                    _sub(dist[:], dst, [[SZ, NL], [M, n]]), op=ALU.add)
            fin = M * (M + 2)
            for mt in range(MT):
                lsub = dpp.tile([P, 1], F32, tag=f"lsub{mt}")
                nc.vector.tensor_tensor(
                    lsub[:], dist[:, 2 * mt * SZ + fin:2 * mt * SZ + fin + 1],
                    dist[:, (2 * mt + 1) * SZ + fin:(2 * mt + 1) * SZ + fin + 1],
                    op=ALU.subtract)
                nc.vector.tensor_scalar(out_tiles[mt][:, 1:2], lsub[:],
                                        MARGIN, 0.0, op0=ALU.add, op1=ALU.max)
                nc.sync.dma_start(out_d[mt * P:(mt + 1) * P, :],
                                  out_tiles[mt][:])
        mine_p.release()
        cst.release()

    nc.compile()
    return nc


_CACHE = {}


def _get_program(cfg):
    if cfg not in _CACHE:
        _CACHE[cfg] = build_program(*cfg)
    return _CACHE[cfg]


def make_in_maps(gf, lf, targets, NC, NCLS=256):
    N, DG = gf.shape
    M, C = lf.shape[1], lf.shape[2]
    NA = N // NC
    MT = NA // P
    CH = C // P
    MC = M * C
    F = M * M
    NL = 2 * MT
    BF = ml_dtypes.bfloat16

    gf = np.asarray(gf, dtype=np.float32)
    t = np.asarray(targets).astype(np.int64)
    gfb = gf.astype(BF)
    gf_mov = np.ascontiguousarray(gfb.T)
    sq = np.einsum('nd,nd->n', gf, gf, dtype=np.float64).astype(np.float32)
    msq = -0.5 * sq
    hi = msq.astype(BF)
    lo = (msq - hi.astype(np.float32)).astype(BF)
    sqhl = np.ascontiguousarray(np.stack([hi, lo]))
    oh_mov = np.zeros((NCLS, N), dtype=BF)
    oh_mov[t, np.arange(N)] = 1.0
    lf32 = np.asarray(lf, dtype=np.float32)
    lfb = lf32.astype(BF)
    sql = np.einsum('nmc,nmc->nm', lf32, lf32).astype(np.float32)
    lfb_flat = np.ascontiguousarray(lfb.reshape(N, MC))

    maps = []
    for c in range(NC):
        sl = slice(c * NA, (c + 1) * NA)
        oh_st = np.zeros((NCLS, NA), dtype=BF)
        oh_st[t[sl], np.arange(NA)] = -0.5 * LBL
        # tx layout: lftx[p, mt*MC + h*M*P + anchor*M + a]
        #          = lf[c*NA + mt*P + anchor, a, h*P + p]
        arr = lfb[sl].reshape(MT, P, M, CH, P)        # [mt, anchor, a, h, p]
        lftx = np.ascontiguousarray(
            arr.transpose(4, 0, 3, 1, 2).reshape(P, MT * MC))
        # sqxp[p, (mt, side, a, b)] = |lf[mt*P+p, a]|^2  (side-duplicated)
        s = sql[sl].reshape(MT, P, M)
        sqxp = np.broadcast_to(s[:, None, :, :, None].transpose(2, 0, 1, 3, 4),
                               (P, MT, 2, M, M)).reshape(P, NL * F)
        maps.append({
            "gf_mov": gf_mov,
            "gf_st": np.ascontiguousarray(gfb[sl].T),
            "oh_mov": oh_mov,
            "oh_st": oh_st,
            "sqhl": sqhl,
            "sq_own": np.ascontiguousarray(sq[sl]),
            "lfb": lfb_flat,
            "lftx": lftx,
            "sql": sql,
            "sqxp": np.ascontiguousarray(sqxp),
        })
    return maps


def kernel(gf, lf, targets):
    NC = 8
    N, DG = gf.shape
    M, C = lf.shape[1], lf.shape[2]
    nc = _get_program((N, DG, M, C, NC, 256))
    in_maps = make_in_maps(gf, lf, targets, NC)
    res = run_bass_kernel_spmd(nc, in_maps, core_ids=list(range(NC)))
    outs = np.concatenate([res.results[c]["out"] for c in range(NC)], axis=0)
    g = outs[:, 0].mean(dtype=np.float64)
    l = outs[:, 1].mean(dtype=np.float64)
    return np.array([g, l], dtype=np.float32)


# revision 25
# speedup vs baseline: 1.2315x; 1.2315x over previous
"""Trainium2 Bass kernel for AlignedTriLoss (global hard-mining triplet loss +
DMLI local-stripe shortest-path loss), SPMD over 8 NeuronCores.

Strategy (row-sharded mining, v4 — dma_gather-transpose DMLI path):
  * Host precomputes everything cheap on CPU: bf16 casts of gf/lf, squared
    norms (-0.5*sq as stacked bf16 hi+lo rows), one-hot label matrices
    (moving: [256,N] 1.0; stationary: [256,NA] -L/2), per-stripe local norms,
    and channel-major pre-transposed own-anchor local features.  No
    AllGather / barrier: each core is fully independent.
  * Each core owns N/8 anchor rows and computes, via a fused bf16 TensorE
    matmul chain with an augmented contraction dimension,
        P[i, j] = gf_i . gf_j  - 0.5*sq_j - 0.5*L*[t_i == t_j]
    so that  -2*P[i, j] + sq_i = d2[i, j] + L*eq[i, j].
    Row min/max of P (+ DVE max_index straight on PSUM) give the hardest
    positive / negative values *and* their column indices.  Columns are
    processed in quarters of 1024; within a quarter the 4 anchor tiles are
    split into two halves so the 8 PSUM banks double-buffer: the PE works on
    one half while the DVE drains the other.
  * DMLI: mined indices are rewrapped (int16, 16-partition layout) with one
    small DMA per anchor tile/side, then a single gpsimd dma_gather with
    transpose=True fetches the 128 hard rows of lf straight into
    channel-major [c, (stripe, chunk), anchor] layout (xbar transpose, no
    PE/DVE work).  Gram matmuls read that tile through strided moving APs
    (16-anchor groups), accumulate over the 2 channel chunks into PSUM, and
    the 8x8 block-diagonals are pulled out with one 4D-AP SBUF->SBUF DMA per
    tile/side.  Stripe norms of the mined rows come from a tiny indirect
    row-gather.  d2 assembly, sqrt+tanh and the 8x8 shortest-path DP run
    batched across all 8 (tile, side) lanes in single wide DVE/ACT ops.
Host side only shards/preprocesses inputs, concatenates per-core outputs and
takes means.
"""

import numpy as np
import ml_dtypes

import concourse.bass as bass
import concourse.bacc as bacc
import concourse.mybir as mybir
import concourse.tile as tile
from concourse.masks import make_identity
from concourse.bass_utils import run_bass_kernel_spmd

F32 = mybir.dt.float32
U32 = mybir.dt.uint32
I16 = mybir.dt.int16
BF16 = mybir.dt.bfloat16
AF = mybir.ActivationFunctionType
ALU = mybir.AluOpType
AX = mybir.AxisListType

P = 128
MARGIN = 0.3
EPS = 1e-12
LBL = 16384.0   # label-match offset; > max d2 (~5k), exact in bf16
BIG = 1e30
BIGI = 65536.0


def _sub(base_ap, off, free_dims):
    """AP at base+off with explicit free dims (keeps base partition dim)."""
    return bass.AP(base_ap.tensor, base_ap.offset + off, [base_ap.ap[0]] + free_dims)


def _flat(base_ap, off, dims):
    """Fully flat AP (dims may cross partitions at the tile's row pitch)."""
    return bass.AP(base_ap.tensor, base_ap.offset + off, dims)


def build_program(N=4096, DG=2048, M=8, C=256, NC=8, NCLS=256):
    NA = N // NC            # anchors per core
    MT = NA // P            # anchor tiles per core
    KG = DG // P            # gf k-tiles
    OH = NCLS // P          # one-hot k-tiles
    CH = C // P             # 128-chunks per local stripe
    G16 = P // (2 * M)      # anchor groups per tile (8 groups of 16)
    NBW = 512               # mining psum tile width
    NQB = 2                 # psum tiles (column blocks) per anchor tile
    QW = NQB * NBW          # columns per quarter
    NQ = N // QW            # quarters
    NPART = NQ * NQB        # per-row partials
    MC = M * C
    NK = KG + OH + 1        # mining k rounds (gf, onehot, stacked sq hi/lo)
    SZ = (M + 1) * (M + 1)  # padded DP matrix size
    F = M * M
    GA = 2 * M              # anchors per gram group (16)
    NL = 2 * MT             # DMLI lanes: (mt, side)
    assert G16 * GA == P and C % P == 0 and MT % 2 == 0

    nc = bacc.Bacc("TRN2", target_bir_lowering=False, debug=False, num_devices=NC)

    gf_mov = nc.dram_tensor("gf_mov", [DG, N], BF16, kind="ExternalInput").ap()
    gf_st = nc.dram_tensor("gf_st", [DG, NA], BF16, kind="ExternalInput").ap()
    oh_mov_d = nc.dram_tensor("oh_mov", [NCLS, N], BF16, kind="ExternalInput").ap()
    oh_st_d = nc.dram_tensor("oh_st", [NCLS, NA], BF16, kind="ExternalInput").ap()
    sqhl_d = nc.dram_tensor("sqhl", [2, N], BF16, kind="ExternalInput").ap()
    sq_own_d = nc.dram_tensor("sq_own", [NA], F32, kind="ExternalInput").ap()
    lfb_d = nc.dram_tensor("lfb", [N, MC], BF16, kind="ExternalInput").ap()
    lftx_d = nc.dram_tensor("lftx", [P, MT * MC], BF16, kind="ExternalInput").ap()
    sql_d = nc.dram_tensor("sql", [N, M], F32, kind="ExternalInput").ap()
    sqxp_d = nc.dram_tensor("sqxp", [P, NL * F], F32, kind="ExternalInput").ap()
    out_d = nc.dram_tensor("out", [NA, 8], F32, kind="ExternalOutput").ap()

    with tile.TileContext(nc) as tc:
        cst = tc.alloc_tile_pool(name="cst", bufs=1)
        mine_p = tc.alloc_tile_pool(name="mine", bufs=1)

        # tiny dummy dma_gather: forces the q7 gather ucode library to load
        # at t~0 (overlapping mining) instead of on the DMLI critical path
        dum_i = cst.tile([P, 8], I16, name="dumi", tag="dumi")
        nc.gpsimd.memset(dum_i[:], 0.0)
        dum_o = cst.tile([P, P], BF16, name="dumo", tag="dumo")
        nc.gpsimd.dma_gather(
            out_ap=_sub(dum_o[:], 0, [[P, 1], [1, P]]),
            in_ap=lfb_d[0:P, 0:P], idxs_ap=dum_i[:],
            num_idxs=P, num_idxs_reg=P, elem_size=P, elem_step=MC,
            transpose=True)

        # ---- stationary mining operands (lead the scalar DGE queue) ----
        st_p0 = tc.alloc_tile_pool(name="st", bufs=KG)
        st = []
        for k in range(KG):
            t = st_p0.tile([P, NA], BF16, tag="st", name=f"st{k}")
            nc.scalar.dma_start(t[:], gf_st[k * P:(k + 1) * P, :])
            st.append(t)
        oh_st = []
        for h in range(OH):
            t = cst.tile([P, NA], BF16, name=f"ohst{h}", tag=f"ohst{h}")
            nc.scalar.dma_start(t[:], oh_st_d[h * P:(h + 1) * P, :])
            oh_st.append(t)
        oh_mov = []
        for h in range(OH):
            t = cst.tile([P, N], BF16, name=f"ohm{h}", tag=f"ohm{h}")
            nc.gpsimd.dma_start(t[:], oh_mov_d[h * P:(h + 1) * P, :])
            oh_mov.append(t)
        sqh = cst.tile([2, N], BF16, name="sqh", tag="sqh")
        nc.gpsimd.dma_start(sqh[:], sqhl_d[:])
        ones2 = cst.tile([2, P], BF16)
        nc.gpsimd.memset(ones2[:], 1.0)
        sqo, txs = [], []
        for mt in range(MT):
            s = cst.tile([P, 1], F32, name=f"sqo{mt}", tag=f"sqo{mt}")
            nc.scalar.dma_start(s[:], sq_own_d[mt * P:(mt + 1) * P])
            sqo.append(s)
            t = cst.tile([P, MC], BF16, name=f"tx{mt}", tag=f"tx{mt}")
            nc.scalar.dma_start(t[:], lftx_d[:, mt * MC:(mt + 1) * MC])
            txs.append(t)
        sqxp = cst.tile([P, NL * F], F32, name="sqxp", tag="sqxp")
        nc.scalar.dma_start(sqxp[:], sqxp_d[:])
        identf = cst.tile([P, P], F32, name="identf", tag="identf")
        make_identity(nc, identf[:])
        sel_all = mine_p.tile([P, NL], F32, name="selall", tag="selall")

        pvq = [mine_p.tile([P, 2 * NPART], F32, tag=f"pvq{mt}", name=f"pvq{mt}")
               for mt in range(MT)]
        piq = [mine_p.tile([P, 2 * NPART], F32, tag=f"piq{mt}", name=f"piq{mt}")
               for mt in range(MT)]

        # ---- mining matmul ----
        with tc.tile_pool(name="minps", bufs=1, space="PSUM") as mmps, \
                tc.tile_pool(name="mov", bufs=KG + 2) as mov_p, \
                tc.tile_pool(name="mq", bufs=4) as mq_p:
            for q in range(NQ):
                qs = q * QW
                movf = []
                for k in range(KG):
                    t = mov_p.tile([P, QW], BF16, tag="movf", name=f"movf{k}")
                    nc.sync.dma_start(t[:], gf_mov[k * P:(k + 1) * P, qs:qs + QW])
                    movf.append(t)
                for half in range(2):
                    mts = (0, 1) if half == 0 else (2, 3)
                    pt = {}
                    for k in range(NK):
                        for mt in mts:
                            if k < KG:
                                lh = st[k][:, mt * P:(mt + 1) * P]
                                rhf = movf[k][:]
                            elif k < KG + OH:
                                lh = oh_st[k - KG][:, mt * P:(mt + 1) * P]
                                rhf = oh_mov[k - KG][:, qs:qs + QW]
                            else:
                                lh = ones2[:]
                                rhf = sqh[:, qs:qs + QW]
                            for nb in range(NQB):
                                if k == 0:
                                    pt[(mt, nb)] = mmps.tile(
                                        [P, NBW], F32, tag=f"mps{mt}_{nb}",
                                        name=f"mps{mt}_{nb}")
                                rh = rhf[:, nb * NBW:(nb + 1) * NBW]
                                nc.tensor.matmul(
                                    pt[(mt, nb)][:], lh, rh,
                                    start=(k == 0), stop=(k == NK - 1))
                    # drain this half's PSUM (overlaps other half's matmuls)
                    for mt in mts:
                        for nb in range(NQB):
                            ix = q * NQB + nb
                            ps = pt[(mt, nb)][:]
                            nc.vector.tensor_reduce(
                                pvq[mt][:, ix:ix + 1], ps, axis=AX.X, op=ALU.min)
                            nc.vector.tensor_reduce(
                                pvq[mt][:, NPART + ix:NPART + ix + 1], ps,
                                axis=AX.X, op=ALU.max)
                            i8a = mq_p.tile([P, 8], U32, tag="i8a", name=f"i8a{mt}")
                            i8b = mq_p.tile([P, 8], U32, tag="i8b", name=f"i8b{mt}")
                            nc.vector.max_index(
                                i8a[:], pvq[mt][:, ix:ix + 1].to_broadcast([P, 8]),
                                ps)
                            nc.vector.max_index(
                                i8b[:],
                                pvq[mt][:, NPART + ix:NPART + ix + 1]
                                .to_broadcast([P, 8]), ps)
                            off = float(qs + nb * NBW)
                            nc.gpsimd.tensor_copy(
                                piq[mt][:, ix:ix + 1], i8a[:, 0:1])
                            nc.gpsimd.tensor_copy(
                                piq[mt][:, NPART + ix:NPART + ix + 1], i8b[:, 0:1])
                            if off:
                                nc.gpsimd.tensor_scalar(
                                    piq[mt][:, ix:ix + 1], piq[mt][:, ix:ix + 1],
                                    off, None, op0=ALU.add)
                                nc.gpsimd.tensor_scalar(
                                    piq[mt][:, NPART + ix:NPART + ix + 1],
                                    piq[mt][:, NPART + ix:NPART + ix + 1],
                                    off, None, op0=ALU.add)

        # ---- combine partials: values, first-occurrence indices, loss ----
        out_tiles, apx_t, anx_t = [], [], []
        for mt in range(MT):
            ot = mine_p.tile([P, 8], F32, name=f"ot{mt}", tag=f"ot{mt}")
            out_tiles.append(ot)
            mn = mine_p.tile([P, 1], F32, tag=f"mn{mt}")
            mx = mine_p.tile([P, 1], F32, tag=f"mx{mt}")
            nc.vector.tensor_reduce(mn[:], pvq[mt][:, 0:NPART], axis=AX.X,
                                    op=ALU.min)
            nc.vector.tensor_reduce(mx[:], pvq[mt][:, NPART:2 * NPART],
                                    axis=AX.X, op=ALU.max)
            idxs = []
            for side, vref in ((0, mn), (1, mx)):
                msk = mine_p.tile([P, NPART], F32, tag=f"msk{mt}_{side}")
                nc.vector.tensor_tensor(
                    msk[:], pvq[mt][:, side * NPART:(side + 1) * NPART],
                    vref[:, 0:1].to_broadcast([P, NPART]), op=ALU.is_equal)
                cnd = mine_p.tile([P, NPART], F32, tag=f"cnd{mt}_{side}")
                nc.vector.tensor_scalar(
                    cnd[:], piq[mt][:, side * NPART:(side + 1) * NPART],
                    -BIGI, None, op0=ALU.add)
                nc.vector.tensor_tensor(cnd[:], cnd[:], msk[:], op=ALU.mult)
                nc.vector.tensor_scalar(cnd[:], cnd[:], BIGI, None, op0=ALU.add)
                lane = mt * 2 + side
                sel = sel_all[:, lane:lane + 1]
                nc.vector.tensor_reduce(sel, cnd[:], axis=AX.X, op=ALU.min)
                iu = mine_p.tile([P, 1], U32, tag=f"iu{mt}_{side}")
                nc.vector.tensor_copy(iu[:], sel)
                nc.gpsimd.tensor_copy(ot[:, 4 + side:5 + side], sel)
                idxs.append(iu)
            apx_t.append(idxs[0])
            anx_t.append(idxs[1])
            # d2_ap = -2*mn - L + sq_i ; d2_an = -2*mx + sq_i
            t1 = mine_p.tile([P, 2], F32, tag=f"t1{mt}")
            nc.vector.tensor_scalar(t1[:, 0:1], mn[:], -2.0, -LBL,
                                    op0=ALU.mult, op1=ALU.add)
            nc.vector.tensor_scalar(t1[:, 1:2], mx[:], -2.0, None, op0=ALU.mult)
            d2 = mine_p.tile([P, 2], F32, tag=f"d2{mt}")
            nc.vector.tensor_tensor(
                d2[:], t1[:], sqo[mt][:, 0:1].to_broadcast([P, 2]), op=ALU.add)
            nc.vector.tensor_scalar(d2[:], d2[:], EPS, None, op0=ALU.max)
            nc.scalar.activation(ot[:, 2:4], d2[:], AF.Sqrt)
            gsub = mine_p.tile([P, 1], F32, tag=f"gs{mt}")
            nc.vector.tensor_tensor(gsub[:], ot[:, 2:3], ot[:, 3:4],
                                    op=ALU.subtract)
            nc.vector.tensor_scalar(ot[:, 0:1], gsub[:], MARGIN, 0.0,
                                    op0=ALU.add, op1=ALU.max)
            nc.gpsimd.tensor_copy(ot[:, 6:7], mn[:])
            nc.gpsimd.tensor_copy(ot[:, 7:8], mx[:])
        st_p0.release()

        # ---- DMLI local loss ----
        with tc.tile_pool(name="dml", bufs=1) as dml, \
                tc.tile_pool(name="gps", bufs=3, space="PSUM") as gps, \
                tc.tile_pool(name="ixp", bufs=1, space="PSUM") as ixp, \
                tc.tile_pool(name="gsb", bufs=4) as gsb_p, \
                tc.tile_pool(name="gdr", bufs=4, space="DRAM") as gdr, \
                tc.tile_pool(name="dp", bufs=1) as dpp:
            dt_all = dml.tile([P, NL * F], BF16, name="dtall", tag="dtall")

            # mined indices -> dma_gather layout: PE-transpose sel_all so
            # each lane's 128 indices land in one partition row, rearrange
            # in-row to the wrapped order, then one 16B-descriptor DMA per
            # lane spreads it to partitions 0-31 of idxw_all
            rp = ixp.tile([NL, P], F32, name="rp", tag="rp")
            nc.tensor.transpose(rp[:], sel_all[:], identf[:])
            r16 = dml.tile([NL, P], I16, name="r16", tag="r16")
            nc.vector.tensor_copy(r16[:], rp[:])
            rw = dml.tile([NL, 2 * P], I16, name="rw", tag="rw")
            nc.vector.tensor_copy(_sub(rw[:], 0, [[8, 16], [1, 8]]),
                                  _sub(r16[:], 0, [[1, 16], [16, 8]]))
            nc.vector.tensor_copy(rw[:, P:2 * P], rw[:, 0:P])
            idxw_all = dml.tile([P, NL * 8], I16, name="ixw", tag="ixw")
            nc.gpsimd.memset(idxw_all[:], 0.0)
            for lane in range(NL):
                (nc.sync if lane % 2 == 0 else nc.scalar).dma_start(
                    _flat(idxw_all[:], lane * 8, [[NL * 8, 32], [1, 8]]),
                    _sub(rw[lane:lane + 1, :], 0, [[8, 32], [1, 8]]))

            # gather+transpose the 128 hard lf rows per lane: ty[c, (b,h), i]
            tys = []
            for lane in range(NL):
                ty = dml.tile([P, MC], BF16, tag=f"ty{lane}",
                              name=f"ty{lane}")
                nc.gpsimd.dma_gather(
                    out_ap=_sub(ty[:], 0, [[P, MC // P], [1, P]]),
                    in_ap=lfb_d[:],
                    idxs_ap=_sub(idxw_all[:], lane * 8, [[1, 8]]),
                    num_idxs=P,
                    num_idxs_reg=P,
                    elem_size=MC,
                    transpose=True,
                )
                tys.append(ty)
            # stripe norms of the mined rows (tiny indirect gathers)
            sqys = {}
            for mt in range(MT):
                for side, iu in ((0, apx_t[mt]), (1, anx_t[mt])):
                    sqy = dpp.tile([P, M], F32, tag=f"sqy{mt}{side}",
                                   name=f"sqy{mt}{side}")
                    nc.gpsimd.indirect_dma_start(
                        out=sqy[:], out_offset=None, in_=sql_d[:],
                        in_offset=bass.IndirectOffsetOnAxis(ap=iu[:, 0:1],
                                                            axis=0))
                    sqys[mt * 2 + side] = sqy

            for lane in range(NL):
                mt, side = lane // 2, lane % 2
                ty = tys[lane]
                # gram matmuls: 8 groups of 16 anchors, accumulate over
                # the CH channel chunks; 4 groups share one PSUM bank
                gpa = gps.tile([P, 4 * P], F32, tag="gramA",
                               name=f"grA{lane}")
                gpb = gps.tile([P, 4 * P], F32, tag="gramB",
                               name=f"grB{lane}")
                for g in range(G16):
                    gp = gpa if g < 4 else gpb
                    col = (g % 4) * P
                    for h in range(CH):
                        nc.tensor.matmul(
                            gp[:, col:col + P],
                            txs[mt][:, h * M * P + g * P:
                                     h * M * P + (g + 1) * P],
                            _sub(ty[:], h * P + g * GA,
                                 [[1, GA], [CH * P, M]]),
                            start=(h == 0), stop=(h == CH - 1))
                gsb = gsb_p.tile([P, G16 * P], BF16, tag="gsb",
                                 name=f"gsb{lane}")
                if side == 0:
                    nc.vector.tensor_copy(gsb[:, 0:4 * P], gpa[:])
                    nc.scalar.activation(gsb[:, 4 * P:8 * P], gpb[:],
                                         AF.Copy)
                else:
                    nc.scalar.activation(gsb[:, 0:4 * P], gpa[:], AF.Copy)
                    nc.vector.tensor_copy(gsb[:, 4 * P:8 * P], gpb[:])
                # bounce the gram tile to DRAM, then per-group 3D DMAs
                # pull the 16 8x8 diag blocks into anchor-major dt:
                # dt[p=(g,u), lane*F + a*M + b]
                gd = gdr.tile([P * G16 * P], BF16, tag="gd",
                              name=f"gd{lane}")
                nc.sync.dma_start(gd[:], gsb[:])
                for g in range(G16):
                    eng = (nc.sync, nc.scalar)[(lane + g) % 2]
                    eng.dma_start(
                        _flat(dt_all[:], (g * GA) * (NL * F) + lane * F,
                              [[NL * F, GA], [M, M], [1, M]]),
                        _flat(gd[:], g * P,
                              [[M * G16 * P + M, GA],
                               [G16 * P, M], [1, M]]))

            # ---- d2 assembly + sqrt/tanh + DP, batched over all 8 lanes ----
            u2 = dpp.tile([P, NL * F], F32, tag="u2", name="u2")
            nc.vector.tensor_scalar(u2[:], dt_all[:], -2.0, None, op0=ALU.mult)
            nc.vector.tensor_tensor(u2[:], u2[:], sqxp[:], op=ALU.add)
            for lane in range(NL):
                nc.vector.tensor_tensor(
                    u2[:, lane * F:(lane + 1) * F],
                    u2[:, lane * F:(lane + 1) * F],
                    _sub(sqys[lane][:], 0, [[0, M], [1, M]]), op=ALU.add)
            nc.vector.tensor_scalar(u2[:], u2[:], EPS, None, op0=ALU.max)
            nc.scalar.activation(u2[:], u2[:], AF.Sqrt)
            dist = dpp.tile([P, NL * SZ], F32, tag="dist", name="dist")
            nc.gpsimd.memset(dist[:], BIG)
            nc.gpsimd.memset(_sub(dist[:], 1, [[SZ, NL], [1, 1]]), 0.0)
            for lane in range(NL):
                nc.scalar.activation(
                    _sub(dist[:], lane * SZ + (M + 1) + 1,
                         [[M + 1, M], [1, M]]),
                    _sub(u2[:], lane * F, [[M, M], [1, M]]),
                    AF.Tanh, scale=0.5)
            tmp = dpp.tile([P, NL * M], F32, tag="dptmp")
            for k in range(2, 2 * M + 1):
                a_lo = max(1, k - M)
                n = min(M, k - 1) - a_lo + 1
                s1 = M * a_lo + k - (M + 1)
                s2 = M * a_lo + k - 1
                dst = M * a_lo + k
                nc.vector.tensor_tensor(
                    _sub(tmp[:], 0, [[M, NL], [1, n]]),
                    _sub(dist[:], s1, [[SZ, NL], [M, n]]),
                    _sub(dist[:], s2, [[SZ, NL], [M, n]]), op=ALU.min)
                nc.vector.tensor_tensor(
                    _sub(dist[:], dst, [[SZ, NL], [M, n]]),
                    _sub(tmp[:], 0, [[M, NL], [1, n]]),
                    _sub(dist[:], dst, [[SZ, NL], [M, n]]), op=ALU.add)
            fin = M * (M + 2)
            for mt in range(MT):
                lsub = dpp.tile([P, 1], F32, tag=f"lsub{mt}")
                nc.vector.tensor_tensor(
                    lsub[:], dist[:, 2 * mt * SZ + fin:2 * mt * SZ + fin + 1],
                    dist[:, (2 * mt + 1) * SZ + fin:(2 * mt + 1) * SZ + fin + 1],
                    op=ALU.subtract)
                nc.vector.tensor_scalar(out_tiles[mt][:, 1:2], lsub[:],
                                        MARGIN, 0.0, op0=ALU.add, op1=ALU.max)
                nc.sync.dma_start(out_d[mt * P:(mt + 1) * P, :],
                                  out_tiles[mt][:])
        mine_p.release()
        cst.release()

    nc.compile()
    return nc


_CACHE = {}


def _get_program(cfg):
    if cfg not in _CACHE:
        _CACHE[cfg] = build_program(*cfg)
    return _CACHE[cfg]


def make_in_maps(gf, lf, targets, NC, NCLS=256):
    N, DG = gf.shape
    M, C = lf.shape[1], lf.shape[2]
    NA = N // NC
    MT = NA // P
    CH = C // P
    MC = M * C
    F = M * M
    NL = 2 * MT
    BF = ml_dtypes.bfloat16

    gf = np.asarray(gf, dtype=np.float32)
    t = np.asarray(targets).astype(np.int64)
    gfb = gf.astype(BF)
    gf_mov = np.ascontiguousarray(gfb.T)
    sq = np.einsum('nd,nd->n', gf, gf, dtype=np.float64).astype(np.float32)
    msq = -0.5 * sq
    hi = msq.astype(BF)
    lo = (msq - hi.astype(np.float32)).astype(BF)
    sqhl = np.ascontiguousarray(np.stack([hi, lo]))
    oh_mov = np.zeros((NCLS, N), dtype=BF)
    oh_mov[t, np.arange(N)] = 1.0
    lf32 = np.asarray(lf, dtype=np.float32)
    lfb = lf32.astype(BF)
    sql = np.einsum('nmc,nmc->nm', lf32, lf32).astype(np.float32)
    lfb_flat = np.ascontiguousarray(lfb.reshape(N, MC))

    maps = []
    for c in range(NC):
        sl = slice(c * NA, (c + 1) * NA)
        oh_st = np.zeros((NCLS, NA), dtype=BF)
        oh_st[t[sl], np.arange(NA)] = -0.5 * LBL
        # tx layout: lftx[p, mt*MC + h*M*P + anchor*M + a]
        #          = lf[c*NA + mt*P + anchor, a, h*P + p]
        arr = lfb[sl].reshape(MT, P, M, CH, P)        # [mt, anchor, a, h, p]
        lftx = np.ascontiguousarray(
            arr.transpose(4, 0, 3, 1, 2).reshape(P, MT * MC))
        # sqxp[p, (mt, side, a, b)] = |lf[mt*P+p, a]|^2  (side-duplicated)
        s = sql[sl].reshape(MT, P, M)
        sqxp = np.broadcast_to(s[:, None, :, :, None].transpose(2, 0, 1, 3, 4),
                               (P, MT, 2, M, M)).reshape(P, NL * F)
        maps.append({
            "gf_mov": gf_mov,
            "gf_st": np.ascontiguousarray(gfb[sl].T),
            "oh_mov": oh_mov,
            "oh_st": oh_st,
            "sqhl": sqhl,
            "sq_own": np.ascontiguousarray(sq[sl]),
            "lfb": lfb_flat,
            "lftx": lftx,
            "sql": sql,
            "sqxp": np.ascontiguousarray(sqxp),
        })
    return maps


def kernel(gf, lf, targets):
    NC = 8
    N, DG = gf.shape
    M, C = lf.shape[1], lf.shape[2]
    nc = _get_program((N, DG, M, C, NC, 256))
    in_maps = make_in_maps(gf, lf, targets, NC)
    res = run_bass_kernel_spmd(nc, in_maps, core_ids=list(range(NC)))
    outs = np.concatenate([res.results[c]["out"] for c in range(NC)], axis=0)
    g = outs[:, 0].mean(dtype=np.float64)
    l = outs[:, 1].mean(dtype=np.float64)
    return np.array([g, l], dtype=np.float32)


# revision 32
# speedup vs baseline: 1.3205x; 1.0723x over previous
"""Trainium2 Bass kernel for AlignedTriLoss (global hard-mining triplet loss +
DMLI local-stripe shortest-path loss), SPMD over 8 NeuronCores.

Strategy (row-sharded mining, v4 — dma_gather-transpose DMLI path):
  * Host precomputes everything cheap on CPU: bf16 casts of gf/lf, squared
    norms (-0.5*sq as stacked bf16 hi+lo rows), one-hot label matrices
    (moving: [256,N] 1.0; stationary: [256,NA] -L/2), per-stripe local norms,
    and channel-major pre-transposed own-anchor local features.  No
    AllGather / barrier: each core is fully independent.
  * Each core owns N/8 anchor rows and computes, via a fused bf16 TensorE
    matmul chain with an augmented contraction dimension,
        P[i, j] = gf_i . gf_j  - 0.5*sq_j - 0.5*L*[t_i == t_j]
    so that  -2*P[i, j] + sq_i = d2[i, j] + L*eq[i, j].
    Row min/max of P (+ DVE max_index straight on PSUM) give the hardest
    positive / negative values *and* their column indices.  Columns are
    processed in quarters of 1024; within a quarter the 4 anchor tiles are
    split into two halves so the 8 PSUM banks double-buffer: the PE works on
    one half while the DVE drains the other.
  * DMLI: mined indices are rewrapped (int16, 16-partition layout) with one
    small DMA per anchor tile/side, then a single gpsimd dma_gather with
    transpose=True fetches the 128 hard rows of lf straight into
    channel-major [c, (stripe, chunk), anchor] layout (xbar transpose, no
    PE/DVE work).  Gram matmuls read that tile through strided moving APs
    (16-anchor groups), accumulate over the 2 channel chunks into PSUM, and
    the 8x8 block-diagonals are pulled out with one 4D-AP SBUF->SBUF DMA per
    tile/side.  Stripe norms of the mined rows come from a tiny indirect
    row-gather.  d2 assembly, sqrt+tanh and the 8x8 shortest-path DP run
    batched across all 8 (tile, side) lanes in single wide DVE/ACT ops.
Host side only shards/preprocesses inputs, concatenates per-core outputs and
takes means.
"""

import numpy as np
import ml_dtypes

import concourse.bass as bass
import concourse.bacc as bacc
import concourse.mybir as mybir
import concourse.tile as tile
from concourse.masks import make_identity
from concourse.bass_utils import run_bass_kernel_spmd

F32 = mybir.dt.float32
U32 = mybir.dt.uint32
I16 = mybir.dt.int16
BF16 = mybir.dt.bfloat16
AF = mybir.ActivationFunctionType
ALU = mybir.AluOpType
AX = mybir.AxisListType

P = 128
MARGIN = 0.3
EPS = 1e-12
LBL = 16384.0   # label-match offset; > max d2 (~5k), exact in bf16
BIG = 1e30
BIGI = 65536.0


def _sub(base_ap, off, free_dims):
    """AP at base+off with explicit free dims (keeps base partition dim)."""
    return bass.AP(base_ap.tensor, base_ap.offset + off, [base_ap.ap[0]] + free_dims)


def _flat(base_ap, off, dims):
    """Fully flat AP (dims may cross partitions at the tile's row pitch)."""
    return bass.AP(base_ap.tensor, base_ap.offset + off, dims)


def build_program(N=4096, DG=2048, M=8, C=256, NC=8, NCLS=256):
    NA = N // NC            # anchors per core
    MT = NA // P            # anchor tiles per core
    KG = DG // P            # gf k-tiles
    OH = NCLS // P          # one-hot k-tiles
    CH = C // P             # 128-chunks per local stripe
    G16 = P // (2 * M)      # anchor groups per tile (8 groups of 16)
    NBW = 512               # mining psum tile width
    NQB = 2                 # psum tiles (column blocks) per anchor tile
    QW = NQB * NBW          # columns per quarter
    NQ = N // QW            # quarters
    NPART = NQ * NQB        # per-row partials
    MC = M * C
    NK = KG + OH + 1        # mining k rounds (gf, onehot, stacked sq hi/lo)
    SZ = (M + 1) * (M + 1)  # padded DP matrix size
    F = M * M
    GA = 2 * M              # anchors per gram group (16)
    NL = 2 * MT             # DMLI lanes: (mt, side)
    assert G16 * GA == P and C % P == 0 and MT % 2 == 0

    nc = bacc.Bacc("TRN2", target_bir_lowering=False, debug=False, num_devices=NC)

    gf_mov = nc.dram_tensor("gf_mov", [DG, N], BF16, kind="ExternalInput").ap()
    gf_st = nc.dram_tensor("gf_st", [DG, NA], BF16, kind="ExternalInput").ap()
    oh_mov_d = nc.dram_tensor("oh_mov", [NCLS, N], BF16, kind="ExternalInput").ap()
    oh_st_d = nc.dram_tensor("oh_st", [NCLS, NA], BF16, kind="ExternalInput").ap()
    sqhl_d = nc.dram_tensor("sqhl", [2, N], BF16, kind="ExternalInput").ap()
    sq_own_d = nc.dram_tensor("sq_own", [NA], F32, kind="ExternalInput").ap()
    lfb_d = nc.dram_tensor("lfb", [N, MC], BF16, kind="ExternalInput").ap()
    lftx_d = nc.dram_tensor("lftx", [P, MT * MC], BF16, kind="ExternalInput").ap()
    sql_d = nc.dram_tensor("sql", [N, M], F32, kind="ExternalInput").ap()
    sqxp_d = nc.dram_tensor("sqxp", [P, NL * F], F32, kind="ExternalInput").ap()
    out_d = nc.dram_tensor("out", [NA, 8], F32, kind="ExternalOutput").ap()

    with tile.TileContext(nc) as tc:
        cst = tc.alloc_tile_pool(name="cst", bufs=1)
        mine_p = tc.alloc_tile_pool(name="mine", bufs=1)

        # tiny dummy dma_gather: forces the q7 gather ucode library to load
        # at t~0 (overlapping mining) instead of on the DMLI critical path
        dum_i = cst.tile([P, 8], I16, name="dumi", tag="dumi")
        nc.gpsimd.memset(dum_i[:], 0.0)
        dum_o = cst.tile([P, P], BF16, name="dumo", tag="dumo")
        nc.gpsimd.dma_gather(
            out_ap=_sub(dum_o[:], 0, [[P, 1], [1, P]]),
            in_ap=lfb_d[0:P, 0:P], idxs_ap=dum_i[:],
            num_idxs=P, num_idxs_reg=P, elem_size=P, elem_step=MC,
            transpose=True)

        # ---- stationary mining operands (lead the scalar DGE queue) ----
        st_p0 = tc.alloc_tile_pool(name="st", bufs=KG)
        st = []
        for k in range(KG):
            t = st_p0.tile([P, NA], BF16, tag="st", name=f"st{k}")
            nc.scalar.dma_start(t[:], gf_st[k * P:(k + 1) * P, :])
            st.append(t)
        oh_st = []
        for h in range(OH):
            t = cst.tile([P, NA], BF16, name=f"ohst{h}", tag=f"ohst{h}")
            nc.scalar.dma_start(t[:], oh_st_d[h * P:(h + 1) * P, :])
            oh_st.append(t)
        oh_mov = []
        for h in range(OH):
            t = cst.tile([P, N], BF16, name=f"ohm{h}", tag=f"ohm{h}")
            nc.gpsimd.dma_start(t[:], oh_mov_d[h * P:(h + 1) * P, :])
            oh_mov.append(t)
        sqh = cst.tile([2, N], BF16, name="sqh", tag="sqh")
        nc.gpsimd.dma_start(sqh[:], sqhl_d[:])
        ones2 = cst.tile([2, P], BF16)
        nc.gpsimd.memset(ones2[:], 1.0)
        sqo, txs = [], []
        for mt in range(MT):
            s = cst.tile([P, 1], F32, name=f"sqo{mt}", tag=f"sqo{mt}")
            nc.scalar.dma_start(s[:], sq_own_d[mt * P:(mt + 1) * P])
            sqo.append(s)
            t = cst.tile([P, MC], BF16, name=f"tx{mt}", tag=f"tx{mt}")
            nc.scalar.dma_start(t[:], lftx_d[:, mt * MC:(mt + 1) * MC])
            txs.append(t)
        sqxp = cst.tile([P, NL * F], F32, name="sqxp", tag="sqxp")
        nc.scalar.dma_start(sqxp[:], sqxp_d[:])
        identf = cst.tile([P, P], F32, name="identf", tag="identf")
        make_identity(nc, identf[:])
        sel_all = mine_p.tile([P, NL], F32, name="selall", tag="selall")
        # index-candidate tiles, pre-filled with the BIGI sentinel so the
        # combine phase only needs a predicated copy + min-reduce
        cnds = []
        for i in range(NL):
            c = mine_p.tile([P, NPART], F32, tag=f"cnd{i}", name=f"cnd{i}")
            nc.gpsimd.memset(c[:], BIGI)
            cnds.append(c)

        pvq = [mine_p.tile([P, 2 * NPART], F32, tag=f"pvq{mt}", name=f"pvq{mt}")
               for mt in range(MT)]
        piq = [mine_p.tile([P, 2 * NPART], F32, tag=f"piq{mt}", name=f"piq{mt}")
               for mt in range(MT)]

        # ---- mining matmul ----
        with tc.tile_pool(name="minps", bufs=1, space="PSUM") as mmps, \
                tc.tile_pool(name="mov", bufs=KG + 2) as mov_p, \
                tc.tile_pool(name="mq", bufs=4) as mq_p:
            for q in range(NQ):
                qs = q * QW
                movf = []
                for k in range(KG):
                    t = mov_p.tile([P, QW], BF16, tag="movf", name=f"movf{k}")
                    nc.sync.dma_start(t[:], gf_mov[k * P:(k + 1) * P, qs:qs + QW])
                    movf.append(t)
                for half in range(2):
                    mts = (0, 1) if half == 0 else (2, 3)
                    pt = {}
                    for k in range(NK):
                        for mt in mts:
                            if k < KG:
                                lh = st[k][:, mt * P:(mt + 1) * P]
                                rhf = movf[k][:]
                            elif k < KG + OH:
                                lh = oh_st[k - KG][:, mt * P:(mt + 1) * P]
                                rhf = oh_mov[k - KG][:, qs:qs + QW]
                            else:
                                lh = ones2[:]
                                rhf = sqh[:, qs:qs + QW]
                            for nb in range(NQB):
                                if k == 0:
                                    pt[(mt, nb)] = mmps.tile(
                                        [P, NBW], F32, tag=f"mps{mt}_{nb}",
                                        name=f"mps{mt}_{nb}")
                                rh = rhf[:, nb * NBW:(nb + 1) * NBW]
                                nc.tensor.matmul(
                                    pt[(mt, nb)][:], lh, rh,
                                    start=(k == 0), stop=(k == NK - 1))
                    # drain this half's PSUM (overlaps other half's matmuls)
                    for mt in mts:
                        for nb in range(NQB):
                            ix = q * NQB + nb
                            ps = pt[(mt, nb)][:]
                            nc.vector.tensor_reduce(
                                pvq[mt][:, ix:ix + 1], ps, axis=AX.X, op=ALU.min)
                            nc.vector.tensor_reduce(
                                pvq[mt][:, NPART + ix:NPART + ix + 1], ps,
                                axis=AX.X, op=ALU.max)
                            i8a = mq_p.tile([P, 8], U32, tag="i8a", name=f"i8a{mt}")
                            i8b = mq_p.tile([P, 8], U32, tag="i8b", name=f"i8b{mt}")
                            nc.vector.max_index(
                                i8a[:], pvq[mt][:, ix:ix + 1].to_broadcast([P, 8]),
                                ps)
                            nc.vector.max_index(
                                i8b[:],
                                pvq[mt][:, NPART + ix:NPART + ix + 1]
                                .to_broadcast([P, 8]), ps)
                            off = float(qs + nb * NBW)
                            nc.gpsimd.tensor_copy(
                                piq[mt][:, ix:ix + 1], i8a[:, 0:1])
                            nc.gpsimd.tensor_copy(
                                piq[mt][:, NPART + ix:NPART + ix + 1], i8b[:, 0:1])
                            if off:
                                nc.gpsimd.tensor_scalar(
                                    piq[mt][:, ix:ix + 1], piq[mt][:, ix:ix + 1],
                                    off, None, op0=ALU.add)
                                nc.gpsimd.tensor_scalar(
                                    piq[mt][:, NPART + ix:NPART + ix + 1],
                                    piq[mt][:, NPART + ix:NPART + ix + 1],
                                    off, None, op0=ALU.add)

        # ---- combine partials: values, first-occurrence indices, loss ----
        out_tiles, apx_t, anx_t = [], [], []
        for mt in range(MT):
            ot = mine_p.tile([P, 8], F32, name=f"ot{mt}", tag=f"ot{mt}")
            out_tiles.append(ot)
            mn = mine_p.tile([P, 1], F32, tag=f"mn{mt}")
            mx = mine_p.tile([P, 1], F32, tag=f"mx{mt}")
            nc.vector.tensor_reduce(mn[:], pvq[mt][:, 0:NPART], axis=AX.X,
                                    op=ALU.min)
            nc.vector.tensor_reduce(mx[:], pvq[mt][:, NPART:2 * NPART],
                                    axis=AX.X, op=ALU.max)
            idxs = []
            for side, vref in ((0, mn), (1, mx)):
                msk = mine_p.tile([P, NPART], mybir.dt.uint8,
                                  tag=f"msk{mt}_{side}")
                nc.vector.tensor_tensor(
                    msk[:], pvq[mt][:, side * NPART:(side + 1) * NPART],
                    vref[:, 0:1].to_broadcast([P, NPART]), op=ALU.is_equal)
                cnd = cnds[mt * 2 + side]
                nc.vector.copy_predicated(
                    cnd[:], msk[:],
                    piq[mt][:, side * NPART:(side + 1) * NPART])
                lane = mt * 2 + side
                sel = sel_all[:, lane:lane + 1]
                nc.vector.tensor_reduce(sel, cnd[:], axis=AX.X, op=ALU.min)
                iu = mine_p.tile([P, 1], U32, tag=f"iu{mt}_{side}")
                nc.gpsimd.tensor_copy(iu[:], sel)
                nc.gpsimd.tensor_copy(ot[:, 4 + side:5 + side], sel)
                idxs.append(iu)
            apx_t.append(idxs[0])
            anx_t.append(idxs[1])
            # d2_ap = -2*mn - L + sq_i ; d2_an = -2*mx + sq_i
            t1 = mine_p.tile([P, 2], F32, tag=f"t1{mt}")
            nc.vector.tensor_scalar(t1[:, 0:1], mn[:], -2.0, -LBL,
                                    op0=ALU.mult, op1=ALU.add)
            nc.vector.tensor_scalar(t1[:, 1:2], mx[:], -2.0, None, op0=ALU.mult)
            d2 = mine_p.tile([P, 2], F32, tag=f"d2{mt}")
            nc.vector.tensor_tensor(
                d2[:], t1[:], sqo[mt][:, 0:1].to_broadcast([P, 2]), op=ALU.add)
            nc.vector.tensor_scalar(d2[:], d2[:], EPS, None, op0=ALU.max)
            nc.scalar.activation(ot[:, 2:4], d2[:], AF.Sqrt)
            gsub = mine_p.tile([P, 1], F32, tag=f"gs{mt}")
            nc.vector.tensor_tensor(gsub[:], ot[:, 2:3], ot[:, 3:4],
                                    op=ALU.subtract)
            nc.vector.tensor_scalar(ot[:, 0:1], gsub[:], MARGIN, 0.0,
                                    op0=ALU.add, op1=ALU.max)
            nc.gpsimd.tensor_copy(ot[:, 6:7], mn[:])
            nc.gpsimd.tensor_copy(ot[:, 7:8], mx[:])
        st_p0.release()

        # ---- DMLI local loss ----
        with tc.tile_pool(name="dml", bufs=1) as dml, \
                tc.tile_pool(name="gps", bufs=3, space="PSUM") as gps, \
                tc.tile_pool(name="ixp", bufs=1, space="PSUM") as ixp, \
                tc.tile_pool(name="gsb", bufs=4) as gsb_p, \
                tc.tile_pool(name="gdr", bufs=4, space="DRAM") as gdr, \
                tc.tile_pool(name="dp", bufs=1) as dpp:
            dt_all = dml.tile([P, NL * F], BF16, name="dtall", tag="dtall")

            # mined indices -> dma_gather layout: PE-transpose sel_all so
            # each lane's 128 indices land in one partition row, rearrange
            # in-row to the wrapped order, then one 16B-descriptor DMA per
            # lane spreads it to partitions 0-31 of idxw_all
            rp = ixp.tile([NL, P], F32, name="rp", tag="rp")
            nc.tensor.transpose(rp[:], sel_all[:], identf[:])
            r16 = dml.tile([NL, P], I16, name="r16", tag="r16")
            nc.vector.tensor_copy(r16[:], rp[:])
            rw = dml.tile([NL, 2 * P], I16, name="rw", tag="rw")
            nc.vector.tensor_copy(_sub(rw[:], 0, [[8, 16], [1, 8]]),
                                  _sub(r16[:], 0, [[1, 16], [16, 8]]))
            nc.vector.tensor_copy(rw[:, P:2 * P], rw[:, 0:P])
            idxw_all = dml.tile([P, NL * 8], I16, name="ixw", tag="ixw")
            nc.gpsimd.memset(idxw_all[:], 0.0)
            for lane in range(NL):
                (nc.sync if lane % 2 == 0 else nc.scalar).dma_start(
                    _flat(idxw_all[:], lane * 8, [[NL * 8, 32], [1, 8]]),
                    _sub(rw[lane:lane + 1, :], 0, [[8, 32], [1, 8]]))

            # gather+transpose the 128 hard lf rows per lane: ty[c, (b,h), i]
            tys = []
            for lane in range(NL):
                ty = dml.tile([P, MC], BF16, tag=f"ty{lane}",
                              name=f"ty{lane}")
                nc.gpsimd.dma_gather(
                    out_ap=_sub(ty[:], 0, [[P, MC // P], [1, P]]),
                    in_ap=lfb_d[:],
                    idxs_ap=_sub(idxw_all[:], lane * 8, [[1, 8]]),
                    num_idxs=P,
                    num_idxs_reg=P,
                    elem_size=MC,
                    transpose=True,
                )
                tys.append(ty)
            # stripe norms of the mined rows (tiny indirect gathers)
            sqy_all = dpp.tile([P, NL * M], F32, tag="sqyall", name="sqyall")
            for mt in range(MT):
                for side, iu in ((0, apx_t[mt]), (1, anx_t[mt])):
                    lane = mt * 2 + side
                    nc.gpsimd.indirect_dma_start(
                        out=sqy_all[:, lane * M:(lane + 1) * M],
                        out_offset=None, in_=sql_d[:],
                        in_offset=bass.IndirectOffsetOnAxis(ap=iu[:, 0:1],
                                                            axis=0))
            # DP buffers (one per 4-lane group), pre-seeded while gathers
            # run.  Layout: cell(i, j, lg) = i*ROWL + lg*(M+1) + j so a
            # whole matrix row of all 4 lanes is contiguous for the scan;
            # the BIG j=0 pad cells double as scan segment barriers.
            ROWL = 4 * (M + 1)
            dists = []
            for grp in range(2):
                dg = dpp.tile([P, (M + 1) * ROWL], F32, tag=f"dist{grp}",
                              name=f"dist{grp}")
                nc.gpsimd.memset(dg[:], BIG)
                nc.gpsimd.memset(_sub(dg[:], 1, [[M + 1, 4], [1, 1]]), 0.0)
                dists.append(dg)
            u2 = dpp.tile([P, NL * F], F32, tag="u2", name="u2")

            for lane in range(NL):
                mt, side = lane // 2, lane % 2
                ty = tys[lane]
                # gram matmuls: 8 groups of 16 anchors, accumulate over
                # the CH channel chunks; 4 groups share one PSUM bank
                gpa = gps.tile([P, 4 * P], F32, tag="gramA",
                               name=f"grA{lane}")
                gpb = gps.tile([P, 4 * P], F32, tag="gramB",
                               name=f"grB{lane}")
                for g in range(G16):
                    gp = gpa if g < 4 else gpb
                    col = (g % 4) * P
                    for h in range(CH):
                        nc.tensor.matmul(
                            gp[:, col:col + P],
                            txs[mt][:, h * M * P + g * P:
                                     h * M * P + (g + 1) * P],
                            _sub(ty[:], h * P + g * GA,
                                 [[1, GA], [CH * P, M]]),
                            start=(h == 0), stop=(h == CH - 1))
                gsb = gsb_p.tile([P, G16 * P], BF16, tag="gsb",
                                 name=f"gsb{lane}")
                if side == 0:
                    nc.vector.tensor_copy(gsb[:, 0:4 * P], gpa[:])
                    nc.scalar.activation(gsb[:, 4 * P:8 * P], gpb[:],
                                         AF.Copy)
                else:
                    nc.scalar.activation(gsb[:, 0:4 * P], gpa[:], AF.Copy)
                    nc.vector.tensor_copy(gsb[:, 4 * P:8 * P], gpb[:])
                # bounce the gram tile to DRAM, then per-group 3D DMAs
                # pull the 16 8x8 diag blocks into anchor-major dt:
                # dt[p=(g,u), lane*F + a*M + b]
                gd = gdr.tile([P * G16 * P], BF16, tag="gd",
                              name=f"gd{lane}")
                nc.sync.dma_start(gd[:], gsb[:])
                for g in range(G16):
                    eng = (nc.sync, nc.scalar, nc.gpsimd)[(lane + g) % 3]
                    eng.dma_start(
                        _flat(dt_all[:], (g * GA) * (NL * F) + lane * F,
                              [[NL * F, GA], [M, M], [1, M]]),
                        _flat(gd[:], g * P,
                              [[M * G16 * P + M, GA],
                               [G16 * P, M], [1, M]]))

            # ---- d2 assembly + sqrt/tanh + scan-DP, 2 groups of 4 lanes ----
            for grp in range(2):
                gl = 4 * grp              # first lane of group
                cs = gl * F               # u2/dt col offset
                dg = dists[grp]
                u2g = u2[:, cs:cs + 4 * F]
                nc.vector.tensor_scalar(u2g, dt_all[:, cs:cs + 4 * F],
                                        -2.0, None, op0=ALU.mult)
                nc.vector.tensor_tensor(u2g, u2g, sqxp[:, cs:cs + 4 * F],
                                        op=ALU.add)
                nc.vector.tensor_tensor(
                    _sub(u2[:], cs, [[F, 4], [M, M], [1, M]]),
                    _sub(u2[:], cs, [[F, 4], [M, M], [1, M]]),
                    _sub(sqy_all[:], gl * M, [[M, 4], [0, M], [1, M]]),
                    op=ALU.add)
                nc.vector.tensor_scalar(u2g, u2g, EPS, None, op0=ALU.max)
                nc.scalar.activation(u2g, u2g, AF.Sqrt)
                for lg in range(4):
                    nc.scalar.activation(
                        _sub(dg[:], ROWL + lg * (M + 1) + 1,
                             [[ROWL, M], [1, M]]),
                        _sub(u2[:], (gl + lg) * F, [[M, M], [1, M]]),
                        AF.Tanh, scale=0.5)
                # min-plus DP: one contiguous scan per matrix row; the BIG
                # j=0 pad cell of each lane kills the carried state at the
                # lane boundary
                for i in range(1, M + 1):
                    nc.vector.tensor_tensor_scan(
                        _sub(dg[:], i * ROWL, [[1, ROWL]]),
                        _sub(dg[:], (i - 1) * ROWL, [[1, ROWL]]),
                        _sub(dg[:], i * ROWL, [[1, ROWL]]),
                        BIG, op0=ALU.min, op1=ALU.add)
                for mt in (2 * grp, 2 * grp + 1):
                    lm = (mt - 2 * grp) * 2
                    fp = M * ROWL + lm * (M + 1) + M
                    fn = M * ROWL + (lm + 1) * (M + 1) + M
                    lsub = dpp.tile([P, 1], F32, tag=f"lsub{mt}")
                    nc.vector.tensor_tensor(
                        lsub[:], dg[:, fp:fp + 1], dg[:, fn:fn + 1],
                        op=ALU.subtract)
                    nc.vector.tensor_scalar(out_tiles[mt][:, 1:2], lsub[:],
                                            MARGIN, 0.0, op0=ALU.add,
                                            op1=ALU.max)
                    nc.sync.dma_start(out_d[mt * P:(mt + 1) * P, :],
                                      out_tiles[mt][:])
        mine_p.release()
        cst.release()

    nc.compile()
    return nc


_CACHE = {}


def _get_program(cfg):
    if cfg not in _CACHE:
        _CACHE[cfg] = build_program(*cfg)
    return _CACHE[cfg]


def make_in_maps(gf, lf, targets, NC, NCLS=256):
    N, DG = gf.shape
    M, C = lf.shape[1], lf.shape[2]
    NA = N // NC
    MT = NA // P
    CH = C // P
    MC = M * C
    F = M * M
    NL = 2 * MT
    BF = ml_dtypes.bfloat16

    gf = np.asarray(gf, dtype=np.float32)
    t = np.asarray(targets).astype(np.int64)
    gfb = gf.astype(BF)
    gf_mov = np.ascontiguousarray(gfb.T)
    sq = np.einsum('nd,nd->n', gf, gf, dtype=np.float64).astype(np.float32)
    msq = -0.5 * sq
    hi = msq.astype(BF)
    lo = (msq - hi.astype(np.float32)).astype(BF)
    sqhl = np.ascontiguousarray(np.stack([hi, lo]))
    oh_mov = np.zeros((NCLS, N), dtype=BF)
    oh_mov[t, np.arange(N)] = 1.0
    lf32 = np.asarray(lf, dtype=np.float32)
    lfb = lf32.astype(BF)
    sql = np.einsum('nmc,nmc->nm', lf32, lf32).astype(np.float32)
    lfb_flat = np.ascontiguousarray(lfb.reshape(N, MC))

    maps = []
    for c in range(NC):
        sl = slice(c * NA, (c + 1) * NA)
        oh_st = np.zeros((NCLS, NA), dtype=BF)
        oh_st[t[sl], np.arange(NA)] = -0.5 * LBL
        # tx layout: lftx[p, mt*MC + h*M*P + anchor*M + a]
        #          = lf[c*NA + mt*P + anchor, a, h*P + p]
        arr = lfb[sl].reshape(MT, P, M, CH, P)        # [mt, anchor, a, h, p]
        lftx = np.ascontiguousarray(
            arr.transpose(4, 0, 3, 1, 2).reshape(P, MT * MC))
        # sqxp[p, (mt, side, a, b)] = |lf[mt*P+p, a]|^2  (side-duplicated)
        s = sql[sl].reshape(MT, P, M)
        sqxp = np.broadcast_to(s[:, None, :, :, None].transpose(2, 0, 1, 3, 4),
                               (P, MT, 2, M, M)).reshape(P, NL * F)
        maps.append({
            "gf_mov": gf_mov,
            "gf_st": np.ascontiguousarray(gfb[sl].T),
            "oh_mov": oh_mov,
            "oh_st": oh_st,
            "sqhl": sqhl,
            "sq_own": np.ascontiguousarray(sq[sl]),
            "lfb": lfb_flat,
            "lftx": lftx,
            "sql": sql,
            "sqxp": np.ascontiguousarray(sqxp),
        })
    return maps


def kernel(gf, lf, targets):
    NC = 8
    N, DG = gf.shape
    M, C = lf.shape[1], lf.shape[2]
    nc = _get_program((N, DG, M, C, NC, 256))
    in_maps = make_in_maps(gf, lf, targets, NC)
    res = run_bass_kernel_spmd(nc, in_maps, core_ids=list(range(NC)))
    outs = np.concatenate([res.results[c]["out"] for c in range(NC)], axis=0)
    g = outs[:, 0].mean(dtype=np.float64)
    l = outs[:, 1].mean(dtype=np.float64)
    return np.array([g, l], dtype=np.float32)
